# revision 19
# baseline (speedup 1.0000x reference)
"""AttnBlock (GroupNorm -> 1x1 qkv -> softmax attention -> 1x1 proj -> residual)
for Trainium2, data-parallel over batch across 8 NeuronCores.

Shapes (hardcoded): x [8, 512, 2048] fp32. One batch element per core.

Per-core algorithm (C=512, L=2048, P=128):
  - GroupNorm: 4 groups of 128 channels; each group is exactly one [128, 2048]
    SBUF tile. Per-partition stats via bn_stats/bn_aggr, cross-partition via
    gpsimd.partition_all_reduce. xn = x*A + B (A = rstd*gn_w, B = gn_b -
    mean*A per channel) written as bf16 into separate tiles; x stays resident
    in fp32 for the final residual.
  - All matmul operands are bf16 (1 cycle/row on the PE vs 4 for fp32);
    accumulation stays fp32 in PSUM. Weights are host-pretransposed AND
    host-converted to bf16, so lhsT = wT[cin, cout] chunks DMA in directly.
  - Q = qw@xn + qb, K likewise, stored channel-major [o(part), l(free)] bf16
    (bias added during the PSUM->SBUF copyback on ScalarE, which also casts).
  - VT = (vw@xn)^T stored [l(part), c(free)] bf16 directly from matmuls
    (lhsT = xn chunk, rhs = vwT chunk). v-bias is folded into the proj bias
    on the host: pb_eff = pb + pw@vb, so VT needs no bias.
  - Attention is computed TRANSPOSED: S^T[j, i] = sum_c K[c,j] Q[c,i] with j
    on partitions, so softmax normalizers d[i] = sum_j exp(S^T) come from
    ones-column matmuls and no transposes of P are ever needed.
    exp (fused with the 1/sqrt(C) scale) runs on ScalarE reading PSUM,
    writing bf16 E tiles. Softmax max-subtraction is skipped (logits here
    are O(1); exp cannot overflow). Normalization by d happens at the very
    end, in fp32, AFTER proj: final = (pw@O_unnorm)*(1/d) + pb_eff + x.
    Since d is summed from the same rounded E used for O, softmax
    weights still sum to exactly 1.
  - O_unnorm[c, i] = sum_j VT[j,c] E[j,i]  (natural matmuls), copyback bf16.
  - dinv = 1/d broadcast across partitions via a K=1 ones matmul (fp32 apply).
"""

import numpy as np

import concourse.bass as bass
import concourse.mybir as mybir
import concourse.tile as tile
from concourse import bass_isa
from concourse.bass_utils import run_bass_kernel_spmd

F32 = mybir.dt.float32
BF16 = mybir.dt.bfloat16

B = 8
C = 512
L = 2048
P = 128
GROUPS = 4
EPS = 1e-6
SCALE = float(C) ** -0.5

NCT = C // P  # 4 channel tiles
NLT = L // P  # 16 L tiles
IB = 512  # i-block width
NIB = L // IB  # 4 i blocks


def build_program():
    from concourse import bacc

    nc = bacc.Bacc("TRN2", target_bir_lowering=False, debug=False, num_devices=B)

    x_d = nc.dram_tensor("x", [C, L], F32, kind="ExternalInput").ap()
    wT_d = {
        p: nc.dram_tensor(f"{p}wT", [C, C], BF16, kind="ExternalInput").ap()
        for p in ("q", "k", "v", "p")
    }
    qb_d = nc.dram_tensor("qb", [C], F32, kind="ExternalInput").ap()
    kb_d = nc.dram_tensor("kb", [C], F32, kind="ExternalInput").ap()
    pb_d = nc.dram_tensor("pb_eff", [C], F32, kind="ExternalInput").ap()
    gnw_d = nc.dram_tensor("gn_w", [C], F32, kind="ExternalInput").ap()
    gnb_d = nc.dram_tensor("gn_b", [C], F32, kind="ExternalInput").ap()
    out_d = nc.dram_tensor("out", [C, L], F32, kind="ExternalOutput").ap()

    from contextlib import ExitStack

    with tile.TileContext(nc) as tc, ExitStack() as ctx:
        _body(ctx, tc, x_d, wT_d, qb_d, kb_d, pb_d, gnw_d, gnb_d, out_d)
    nc.compile()
    return nc


def _body(ctx, tc, x_d, wT_d, qb_d, kb_d, pb_d, gnw_d, gnb_d, out_d):
    nc = tc.nc
    Exp = mybir.ActivationFunctionType.Exp
    Identity = mybir.ActivationFunctionType.Identity
    Sqrt = mybir.ActivationFunctionType.Sqrt
    mult = mybir.AluOpType.mult
    add = mybir.AluOpType.add

    consts = ctx.enter_context(tc.tile_pool(name="consts", bufs=1))
    persist = ctx.enter_context(tc.tile_pool(name="persist", bufs=1))
    xe_pool = ctx.enter_context(tc.tile_pool(name="xe", bufs=8))
    small = ctx.enter_context(tc.tile_pool(name="small", bufs=4))
    osb_pool = ctx.enter_context(tc.tile_pool(name="osb", bufs=8))
    fin_pool = ctx.enter_context(tc.tile_pool(name="fin", bufs=4))
    dinv_pool = ctx.enter_context(tc.tile_pool(name="dinv", bufs=2))
    ps_pool = ctx.enter_context(tc.tile_pool(name="ps", bufs=4, space="PSUM"))
    psd_pool = ctx.enter_context(tc.tile_pool(name="psd", bufs=2, space="PSUM"))
    psb_pool = ctx.enter_context(tc.tile_pool(name="psb", bufs=2, space="PSUM"))

    # ---- constants ----
    wT = {}
    for p in ("q", "k", "v", "p"):
        for cc in range(NCT):
            t = consts.tile([P, C], BF16, name=f"wT_{p}_{cc}", tag=f"wT_{p}_{cc}")
            nc.sync.dma_start(out=t, in_=wT_d[p][cc * P : (cc + 1) * P, :])
            wT[(p, cc)] = t

    def load_cvec(name, src):
        t = consts.tile([P, NCT], F32, name=name, tag=name)
        for ct in range(NCT):
            nc.sync.dma_start(out=t[:, ct : ct + 1], in_=src[ct * P : (ct + 1) * P, None])
        return t

    qb_sb = load_cvec("qb_sb", qb_d)
    kb_sb = load_cvec("kb_sb", kb_d)
    pb_sb = load_cvec("pb_sb", pb_d)
    gnw_sb = load_cvec("gnw_sb", gnw_d)
    gnb_sb = load_cvec("gnb_sb", gnb_d)

    ones_col = consts.tile([P, 1], BF16, name="ones_col", tag="ones_col")
    nc.vector.memset(ones_col, 1.0)
    ones_col_f32 = consts.tile([P, 1], F32, name="ones_col_f32", tag="ones_col_f32")
    nc.vector.memset(ones_col_f32, 1.0)
    ones_row_f32 = consts.tile([1, P], F32, name="ones_row_f32", tag="ones_row_f32")
    nc.vector.memset(ones_row_f32, 1.0)
    eps_t = consts.tile([P, 1], F32, name="eps_t", tag="eps_t")
    nc.vector.memset(eps_t, EPS)

    # ---- load x (stays resident, fp32) + groupnorm into bf16 xn tiles ----
    x_sb = []
    for g in range(GROUPS):
        xg = persist.tile([P, L], F32, name=f"x_{g}", tag=f"x_{g}")
        nc.sync.dma_start(out=xg, in_=x_d[g * P : (g + 1) * P, :])
        x_sb.append(xg)

    xn = []
    for g in range(GROUPS):
        xg = x_sb[g]
        stats = small.tile([P, 4, 6], F32, name=f"gnstats_{g}", tag=f"gnstats_{g}", bufs=1)
        for s in range(4):
            nc.vector.bn_stats(out=stats[:, s, :], in_=xg[:, s * 512 : (s + 1) * 512])
        mv = small.tile([P, 2], F32, name=f"gnmv_{g}", tag=f"gnmv_{g}", bufs=1)
        nc.vector.bn_aggr(out=mv, in_=stats)
        # mv = [mean_p, var_p] per partition; mv[:,1] <- var_p + mean_p^2
        nc.vector.scalar_tensor_tensor(
            out=mv[:, 1:2], in0=mv[:, 0:1], scalar=mv[:, 0:1], in1=mv[:, 1:2],
            op0=mult, op1=add,
        )
        # cross-partition sum of [mean_p, m2_p] via exact fp32 ones-matmuls:
        # [128,2] -> [1,2] (reduce) -> [128,2] (broadcast)
        gsum_ps = psd_pool.tile([1, 2], F32, tag="d", name=f"gsum_ps_{g}")
        nc.tensor.matmul(gsum_ps, lhsT=ones_col_f32, rhs=mv, start=True, stop=True)
        gsum = small.tile([1, 2], F32, name=f"gsum_{g}", tag=f"gsum_{g}", bufs=1)
        nc.scalar.copy(gsum, gsum_ps)
        gbc_ps = psb_pool.tile([P, 2], F32, tag="db", name=f"gbc_ps_{g}")
        nc.tensor.matmul(gbc_ps, lhsT=ones_row_f32, rhs=gsum, start=True, stop=True)
        nc.scalar.copy(mv, gbc_ps)
        nc.vector.tensor_scalar_mul(mv, mv, 1.0 / P)  # [mean_g, E[x^2]_g]
        msq = small.tile([P, 1], F32, name=f"gnmsq_{g}", tag=f"gnmsq_{g}", bufs=1)
        nc.vector.tensor_mul(msq, mv[:, 0:1], mv[:, 0:1])
        varg = small.tile([P, 1], F32, name=f"gnvar_{g}", tag=f"gnvar_{g}", bufs=1)
        nc.vector.tensor_sub(varg, mv[:, 1:2], msq)
        stdg = small.tile([P, 1], F32, name=f"gnstd_{g}", tag=f"gnstd_{g}", bufs=1)
        nc.scalar.activation(stdg, varg, Sqrt, bias=eps_t)
        rstd = small.tile([P, 1], F32, name=f"gnrstd_{g}", tag=f"gnrstd_{g}", bufs=1)
        nc.vector.reciprocal(rstd, stdg)
        a_t = small.tile([P, 1], F32, name=f"gnA_{g}", tag=f"gnA_{g}", bufs=1)
        nc.vector.tensor_mul(a_t, rstd, gnw_sb[:, g : g + 1])
        ma_t = small.tile([P, 1], F32, name=f"gnmA_{g}", tag=f"gnmA_{g}", bufs=1)
        nc.vector.tensor_mul(ma_t, mv[:, 0:1], a_t)
        b_t = small.tile([P, 1], F32, name=f"gnB_{g}", tag=f"gnB_{g}", bufs=1)
        nc.vector.tensor_sub(b_t, gnb_sb[:, g : g + 1], ma_t)
        # xn = bf16(x*A + B)
        xng = xe_pool.tile([P, L], BF16, tag="xe", name=f"xn_{g}")
        nc.vector.tensor_scalar(
            out=xng, in0=xg, scalar1=a_t, scalar2=b_t, op0=mult, op1=add
        )
        xn.append(xng)

    # ---- Q, K [o(part), l] bf16 ; VT [l(part), c] bf16 ----
    q_sb, k_sb = [], []
    for ot in range(NCT):
        for pname, dest, bias in (("q", q_sb, qb_sb), ("k", k_sb, kb_sb)):
            t = persist.tile([P, L], BF16, name=f"{pname}_{ot}", tag=f"{pname}_{ot}")
            for lb in range(NIB):
                ps = ps_pool.tile([P, IB], F32, tag="ps", name=f"qk_ps_{pname}_{ot}_{lb}")
                for cc in range(NCT):
                    nc.tensor.matmul(
                        ps,
                        lhsT=wT[(pname, cc)][:, ot * P : (ot + 1) * P],
                        rhs=xn[cc][:, lb * IB : (lb + 1) * IB],
                        start=(cc == 0),
                        stop=(cc == NCT - 1),
                    )
                nc.scalar.activation(
                    t[:, lb * IB : (lb + 1) * IB], ps, Identity,
                    bias=bias[:, ot : ot + 1],
                )
            dest.append(t)

    vt_sb = []
    for lt in range(NLT):
        t = persist.tile([P, C], BF16, name=f"vt_{lt}", tag=f"vt_{lt}")
        ps = ps_pool.tile([P, C], F32, tag="ps", name=f"vt_ps_{lt}")
        for cc in range(NCT):
            nc.tensor.matmul(
                ps,
                lhsT=xn[cc][:, lt * P : (lt + 1) * P],
                rhs=wT[("v", cc)],
                start=(cc == 0),
                stop=(cc == NCT - 1),
            )
        nc.scalar.copy(t, ps)
        vt_sb.append(t)

    # ---- attention, i-block at a time ----
    for ib in range(NIB):
        isl = slice(ib * IB, (ib + 1) * IB)

        # E = exp(scale * K^T Q) bf16, transposed layout [j(part), i], packed
        # as 4 tiles [128, 2048] holding 4 j-tiles each.
        e_pack = [
            xe_pool.tile([P, L], BF16, tag="xe", name=f"e_{ib}_{t}") for t in range(4)
        ]

        def e_view(jt):
            t, s = divmod(jt, 4)
            return e_pack[t][:, s * IB : (s + 1) * IB]

        for jt in range(NLT):
            ps = ps_pool.tile([P, IB], F32, tag="ps", name=f"s_ps_{ib}_{jt}")
            for cc in range(NCT):
                nc.tensor.matmul(
                    ps,
                    lhsT=k_sb[cc][:, jt * P : (jt + 1) * P],
                    rhs=q_sb[cc][:, isl],
                    start=(cc == 0),
                    stop=(cc == NCT - 1),
                )
            nc.scalar.activation(e_view(jt), ps, Exp, scale=SCALE)

        # d[i] = sum_j E[j, i]  (ones-column matmuls, M=1)
        d_ps = psd_pool.tile([1, IB], F32, tag="d", name=f"d_ps_{ib}")
        for jt in range(NLT):
            nc.tensor.matmul(
                d_ps,
                lhsT=ones_col,
                rhs=e_view(jt),
                start=(jt == 0),
                stop=(jt == NLT - 1),
            )
        dinv = dinv_pool.tile([1, IB], F32, tag="dinv", name=f"dinv_{ib}")
        nc.vector.reciprocal(dinv, d_ps)
        # broadcast 1/d across partitions with an exact fp32 K=1 ones-matmul
        dinvb_ps = psb_pool.tile([P, IB], F32, tag="db", name=f"dinvb_ps_{ib}")
        nc.tensor.matmul(
            dinvb_ps, lhsT=ones_row_f32, rhs=dinv, start=True, stop=True
        )
        dinvb = dinv_pool.tile([P, IB], F32, tag="dinvb", name=f"dinvb_{ib}")
        nc.scalar.copy(dinvb, dinvb_ps)

        # O_unnorm[c, i] = sum_j VT[j, c] E[j, i]
        o_sb = []
        for ct in range(NCT):
            ps = ps_pool.tile([P, IB], F32, tag="ps", name=f"o_ps_{ib}_{ct}")
            for jt in range(NLT):
                nc.tensor.matmul(
                    ps,
                    lhsT=vt_sb[jt][:, ct * P : (ct + 1) * P],
                    rhs=e_view(jt),
                    start=(jt == 0),
                    stop=(jt == NLT - 1),
                )
            ot_t = osb_pool.tile([P, IB], BF16, tag="osb", name=f"osb_{ib}_{ct}")
            nc.scalar.copy(ot_t, ps)
            o_sb.append(ot_t)

        # out2 = pw @ O_unnorm ; final = out2*dinv + pb_eff + x
        for ot in range(NCT):
            ps2 = ps_pool.tile([P, IB], F32, tag="ps", name=f"p_ps_{ib}_{ot}")
            for cc in range(NCT):
                nc.tensor.matmul(
                    ps2,
                    lhsT=wT[("p", cc)][:, ot * P : (ot + 1) * P],
                    rhs=o_sb[cc],
                    start=(cc == 0),
                    stop=(cc == NCT - 1),
                )
            t1 = fin_pool.tile([P, IB], F32, tag="t1", name=f"t1_{ib}_{ot}")
            nc.vector.tensor_mul(t1, ps2, dinvb)
            fo = fin_pool.tile([P, IB], F32, tag="fo", name=f"fo_{ib}_{ot}")
            nc.vector.scalar_tensor_tensor(
                out=fo, in0=t1, scalar=pb_sb[:, ot : ot + 1],
                in1=x_sb[ot][:, isl], op0=add, op1=add,
            )
            nc.sync.dma_start(out=out_d[ot * P : (ot + 1) * P, isl], in_=fo)


_NC_CACHE = None


def _get_program():
    global _NC_CACHE
    if _NC_CACHE is None:
        _NC_CACHE = build_program()
    return _NC_CACHE


def make_in_maps(x, gn_w, gn_b, qw, qb, kw, kb, vw, vb, pw, pb):
    import ml_dtypes

    f = np.float32
    bf = ml_dtypes.bfloat16
    qwT = np.ascontiguousarray(np.asarray(qw, f).T.astype(bf))
    kwT = np.ascontiguousarray(np.asarray(kw, f).T.astype(bf))
    vwT = np.ascontiguousarray(np.asarray(vw, f).T.astype(bf))
    pwT = np.ascontiguousarray(np.asarray(pw, f).T.astype(bf))
    pb_eff = np.asarray(pb, f) + np.asarray(pw, f) @ np.asarray(vb, f)
    shared = {
        "qwT": qwT, "kwT": kwT, "vwT": vwT, "pwT": pwT,
        "qb": np.ascontiguousarray(np.asarray(qb, f)),
        "kb": np.ascontiguousarray(np.asarray(kb, f)),
        "pb_eff": np.ascontiguousarray(pb_eff),
        "gn_w": np.ascontiguousarray(np.asarray(gn_w, f)),
        "gn_b": np.ascontiguousarray(np.asarray(gn_b, f)),
    }
    x = np.asarray(x, f)
    return [{"x": np.ascontiguousarray(x[b]), **shared} for b in range(B)]


def kernel(x, gn_w, gn_b, qw, qb, kw, kb, vw, vb, pw, pb):
    nc = _get_program()
    in_maps = make_in_maps(x, gn_w, gn_b, qw, qb, kw, kb, vw, vb, pw, pb)
    res = run_bass_kernel_spmd(nc, in_maps, core_ids=list(range(B)))
    return np.stack([res.results[b]["out"] for b in range(B)]).astype(np.float32)
